# revision 15
# baseline (speedup 1.0000x reference)
"""Trainium2 Bass kernel for Enhanced Physics Attention with Sonata.

Contract: kernel(**inputs) takes FULL unsharded numpy inputs (as produced by
setup_inputs()) and returns the FULL [B, N, C] output. Internally shards
across 8 NeuronCores: core c handles batch c//2, token half c%2, and sonata
half c%2 (cross-attention partials). Two pairwise AllReduces: pooled slice
tokens after phase A, cross-attention numerators/denominators in phase B.

Math (validated vs reference, rel err ~1e-6, fp32 throughout):
  Host folds Wslice+temperature into Wxs (s = x @ Wxs), SCALE into Wq/Wck.
  Phase A (token-major): e = exp(s) [logits bounded ~3, no max needed],
    Z = grouped sum, pooling slice_token = e^T @ (fx * 1/Z | 1/Z) — the
    appended 1/Z column yields slice_norm for free.
  Phase B: slice self-attention (replicated) + sonata cross-attention over
    this core's sonata half; ones-column gives the softmax denominator, the
    pair AllReduce sums numerator+denominator over the full sonata.
  Phase C: out = e_hat @ (out_slice @ Wo), using PE-transposed e_hat kept
    half in SBUF / half spilled to DRAM.

Self-contained: hardcodes all shapes; does not read sibling files.
"""

import contextlib
import sys

try:
    import concourse  # noqa: F401
except ImportError:
    sys.path.insert(0, "/opt/trn_rl_repo")

import numpy as np

import concourse.bass as bass
import concourse.tile as tile
from concourse import bacc, mybir
from concourse.bass_utils import run_bass_kernel_spmd
from concourse.masks import make_identity

F32 = mybir.dt.float32
F32R = mybir.dt.float32r
AF = mybir.ActivationFunctionType

# Problem shapes
B, N, C = 4, 16384, 256
H, D, G = 8, 64, 64
M, SD = 1024, 768
SCALE = D**-0.5
HG = H * G  # 512
HD = H * D  # 512
MH = M // 2  # sonata tokens per core (m-split across the pair)


def _bcast_ap(dram_ap: bass.AP, parts: int, n: int, offset: int = 0) -> bass.AP:
    """AP reading n contiguous DRAM floats, replicated across `parts` partitions."""
    return bass.AP(
        tensor=dram_ap.tensor,
        offset=dram_ap.offset + offset,
        ap=[[0, parts], [1, n]],
    )


def _fbcast(ap2d: bass.AP, rep: int) -> bass.AP:
    """[p, k] AP -> [p, k, rep] with step-0 innermost broadcast."""
    return bass.AP(tensor=ap2d.tensor, offset=ap2d.offset,
                   ap=[ap2d.ap[0], ap2d.ap[1], [0, rep]])


def _build(n_cores: int, T: int, flags: dict, no_collective: bool = False):
    """Build the per-core Bass module. T = tokens per core (multiple of 1024)."""
    assert T % 1024 == 0
    NSUP = T // 1024  # super-tiles (1 MiB x loads)
    NT = T // 128  # 128-token tiles
    RES = NT // 2  # tiles whose e-hat^T stays resident in SBUF

    nc = bacc.Bacc(
        "TRN2", target_bir_lowering=False, debug=False, num_devices=n_cores
    )

    # ---- DRAM I/O ----
    xt = nc.dram_tensor("xt", [C, T], F32, kind="ExternalInput")  # x slice, transposed
    snt = nc.dram_tensor("snt", [SD, MH], F32, kind="ExternalInput")  # sonata half, T
    wxs = nc.dram_tensor("wxs", [C, HG], F32, kind="ExternalInput")
    wfx = nc.dram_tensor("wfx", [C, HD], F32, kind="ExternalInput")
    wsp = nc.dram_tensor("wsp", [SD, HD], F32, kind="ExternalInput")
    wq = nc.dram_tensor("wq", [D, D], F32, kind="ExternalInput")
    wk = nc.dram_tensor("wk", [D, D], F32, kind="ExternalInput")
    wv = nc.dram_tensor("wv", [D, D], F32, kind="ExternalInput")
    wck = nc.dram_tensor("wck", [D, D], F32, kind="ExternalInput")
    wcv = nc.dram_tensor("wcv", [D, D], F32, kind="ExternalInput")
    wo = nc.dram_tensor("wo", [HD, C], F32, kind="ExternalInput")
    bqv = nc.dram_tensor("bqv", [3, D], F32, kind="ExternalInput")  # bq,bk,bv rows
    bck = nc.dram_tensor("bck", [D], F32, kind="ExternalInput")
    bsp = nc.dram_tensor("bsp", [HD], F32, kind="ExternalInput")
    if flags["bias_s"]:
        bias_s = nc.dram_tensor("bias_s", [HG], F32, kind="ExternalInput")
    if flags["bfx"]:
        bfx = nc.dram_tensor("bfx", [HD], F32, kind="ExternalInput")
    if flags["bcv"]:
        bcv = nc.dram_tensor("bcv", [D], F32, kind="ExternalInput")
    if flags["bo"]:
        bo = nc.dram_tensor("bo", [C], F32, kind="ExternalInput")
    y = nc.dram_tensor("y", [T, C], F32, kind="ExternalOutput")

    groups = [[2 * i, 2 * i + 1] for i in range(n_cores // 2)]

    def all_reduce(ar_in, ar_out):
        if no_collective:
            nc.gpsimd.dma_start(out=ar_out[:], in_=ar_in[:])
        else:
            nc.gpsimd.collective_compute(
                "AllReduce", mybir.AluOpType.add, replica_groups=groups,
                ins=[ar_in.opt()], outs=[ar_out.opt()])

    with tile.TileContext(nc) as tc, contextlib.ExitStack() as top:
        singles = top.enter_context(tc.tile_pool(name="singles", bufs=1))
        dram = top.enter_context(tc.tile_pool(name="dram", bufs=1, space="DRAM"))

        # ---- resident weights / inputs ----
        # Emission order matters: the in-order DMA queues must deliver wxs/wfx
        # and the first x super-tile before anything else so PE starts ASAP.
        wxs_sb = singles.tile([128, 2, HG], F32)
        wfx_sb = singles.tile([128, 2, HD], F32)
        wxs_re = wxs.ap().rearrange("(k p) n -> p k n", p=128)
        wfx_re = wfx.ap().rearrange("(k p) n -> p k n", p=128)
        # k0 chunks first so the very first matmuls can start sooner
        nc.sync.dma_start(out=wxs_sb[:, 0, :], in_=wxs_re[:, 0, :])
        nc.sync.dma_start(out=wfx_sb[:, 0, :], in_=wfx_re[:, 0, :])
        nc.sync.dma_start(out=wxs_sb[:, 1, :], in_=wxs_re[:, 1, :])
        nc.sync.dma_start(out=wfx_sb[:, 1, :], in_=wfx_re[:, 1, :])
        ident = singles.tile([128, 128], F32)
        make_identity(nc, ident[:])

        # tiles for deferred loads (DMAs emitted after the first super-tile)
        snt_sb = singles.tile([128, 6, MH], F32)
        wsp_sb = singles.tile([128, 6, HD], F32)
        w64 = singles.tile([64, 5, D], F32)  # wq,wk,wv,wck,wcv
        wo_sb = singles.tile([64, H, C], F32)

        def load_phaseB_weights():
            nc.sync.dma_start(out=snt_sb[:],
                              in_=snt.ap().rearrange("(k p) m -> p k m", p=128))
            nc.sync.dma_start(out=wsp_sb[:],
                              in_=wsp.ap().rearrange("(k p) n -> p k n", p=128))
            for idx, w in enumerate((wq, wk, wv, wck, wcv)):
                nc.sync.dma_start(out=w64[:, idx, :], in_=w.ap())
            nc.sync.dma_start(out=wo_sb[:],
                              in_=wo.ap().rearrange("(h d) c -> d h c", d=64))

        # e-hat^T: chunk c holds heads 2c,2c+1 stacked on partitions.
        eT = singles.tile([128, 4, RES * 128], F32)  # resident half
        eTsp = dram.tile([128, NT - RES, 4, 128], F32)  # spilled half, tile-major
        # OS (slice-token outputs @ Wo), chunk-packed like eT
        os_sb = singles.tile([128, 4, C], F32)

        # small per-partition bias columns
        bqv_c = singles.tile([64, 3], F32)
        nc.sync.dma_start(out=bqv_c[:], in_=bqv.ap().rearrange("q d -> d q"))
        bck_c = singles.tile([64, 1], F32)
        nc.sync.dma_start(out=bck_c[:], in_=bck.ap().rearrange("(d o) -> d o", o=1))
        bsp_c = singles.tile([128, 4], F32)
        nc.sync.dma_start(out=bsp_c[:], in_=bsp.ap().rearrange("(k p) -> p k", p=128))

        if flags["bias_s"]:
            bias_s_bc = singles.tile([128, HG], F32)
            nc.sync.dma_start(out=bias_s_bc[:], in_=_bcast_ap(bias_s.ap(), 128, HG))
        if flags["bcv"]:
            bcv_bc = singles.tile([128, D], F32)
            nc.sync.dma_start(out=bcv_bc[:], in_=_bcast_ap(bcv.ap(), 128, D))
        if flags["bo"]:
            bo_bc = singles.tile([64, C], F32)
            nc.sync.dma_start(out=bo_bc[:], in_=_bcast_ap(bo.ap(), 64, C))
        if flags["bfx"]:
            bfx_bc = singles.tile([128, D], F32)

        # ---------------- Phase A ----------------
        with contextlib.ExitStack() as phA:
            pA = phA.enter_context(tc.tile_pool(name="pA", bufs=3))
            pAx = phA.enter_context(tc.tile_pool(name="pAx", bufs=2))
            psA = phA.enter_context(tc.tile_pool(name="psA", bufs=2, space="PSUM"))
            psAcc = phA.enter_context(tc.tile_pool(name="psAcc", bufs=1, space="PSUM"))

            # head h: partitions (h%2)*64, accumulator tile h//2. Each accumulator
            # gets its own PSUM bank (own 2KB zero region) so the start=True of one
            # head pair cannot mark another pair's freshly-written bytes pending-zero.
            pool_ps = [psAcc.tile([128, 128], F32, name=f"poolacc{s}", tag=f"acc{s}")
                       for s in range(4)]

            xt_re = xt.ap().rearrange("(k p) n -> p k n", p=128)
            for si in range(NSUP):
                xt_sb = pAx.tile([128, 2, 1024], F32, tag="xt")
                sl = slice(si * 1024, (si + 1) * 1024)
                if si == 0:
                    nc.sync.dma_start(out=xt_sb[:, 0, :], in_=xt_re[:, 0, sl])
                    nc.sync.dma_start(out=xt_sb[:, 1, :], in_=xt_re[:, 1, sl])
                else:
                    nc.sync.dma_start(out=xt_sb[:], in_=xt_re[:, :, sl])
                for j in range(8):
                    i = si * 8 + j
                    tok = slice(j * 128, (j + 1) * 128)
                    s_ps = psA.tile([128, HG], F32, tag="s", bufs=1)
                    nc.tensor.matmul(s_ps[:], lhsT=xt_sb[:, 0, tok],
                                     rhs=wxs_sb[:, 0, :], start=True, stop=False)
                    nc.tensor.matmul(s_ps[:], lhsT=xt_sb[:, 1, tok],
                                     rhs=wxs_sb[:, 1, :], start=False, stop=True)
                    fx_ps = psA.tile([128, HD], F32, tag="fx")
                    nc.tensor.matmul(fx_ps[:], lhsT=xt_sb[:, 0, tok],
                                     rhs=wfx_sb[:, 0, :], start=True, stop=False)
                    nc.tensor.matmul(fx_ps[:], lhsT=xt_sb[:, 1, tok],
                                     rhs=wfx_sb[:, 1, :], start=False, stop=True)

                    e = pA.tile([128, H, G], F32, tag="e")
                    ef = e.rearrange("p a b -> p (a b)")
                    if flags["bias_s"]:
                        s_sb = pA.tile([128, HG], F32, tag="ssb")
                        nc.vector.tensor_add(out=s_sb[:], in0=s_ps[:], in1=bias_s_bc[:])
                        nc.scalar.activation(out=ef, in_=s_sb[:], func=AF.Exp)
                    else:
                        nc.scalar.activation(out=ef, in_=s_ps[:], func=AF.Exp)

                    z = pA.tile([128, H], F32, tag="z")
                    nc.vector.reduce_sum(out=z[:], in_=e[:], axis=mybir.AxisListType.X)
                    zr = pA.tile([128, H], F32, tag="zr")
                    nc.vector.reciprocal(out=zr[:], in_=z[:])
                    zrb = _fbcast(zr[:], G)  # [128, 8, (0x64)]

                    eh = pA.tile([128, H, G], F32, tag="eh")
                    nc.vector.tensor_tensor(out=eh[:], in0=e[:], in1=zrb,
                                            op=mybir.AluOpType.mult)
                    fxe = pA.tile([128, H, D + 1], F32, tag="fxe")
                    nc.vector.tensor_tensor(
                        out=fxe[:, :, 0:D],
                        in0=fx_ps.rearrange("p (a b) -> p a b", a=H), in1=zrb,
                        op=mybir.AluOpType.mult)
                    nc.vector.tensor_copy(out=fxe[:, :, D], in_=zr[:])

                    for h in range(H):
                        nc.tensor.matmul(
                            pool_ps[h // 2][(h % 2) * 64 : (h % 2) * 64 + 64, 0 : D + 1],
                            lhsT=e[:, h, :], rhs=fxe[:, h, :],
                            start=(i == 0), stop=(i == NT - 1), skip_group_check=True)

                    etp = psA.tile([128, 4, 128], F32, tag="etp", bufs=1)
                    ehf = eh.rearrange("p a b -> p (a b)")
                    for cc in range(4):
                        nc.tensor.transpose(etp[:, cc, :],
                                            ehf[:, cc * 128 : (cc + 1) * 128],
                                            ident[:])
                    if i < RES:
                        nc.scalar.activation(
                            out=eT[:, :, i * 128 : (i + 1) * 128], in_=etp[:],
                            func=AF.Copy)
                    else:
                        sp = pA.tile([128, 4, 128], F32, tag="spill")
                        nc.scalar.activation(out=sp[:], in_=etp[:], func=AF.Copy)
                        nc.sync.dma_start(out=eTsp[:, i - RES, :, :], in_=sp[:])

                if si == 0:
                    load_phaseB_weights()

            # pooled partials -> AllReduce across the pair
            pool_sb = pA.tile([128, 4, D + 1], F32, tag="poolsb")
            for s in range(4):
                nc.scalar.activation(out=pool_sb[:, s, :],
                                     in_=pool_ps[s][:, 0 : D + 1], func=AF.Copy)
            ar_in = dram.tile([128, 4 * (D + 1)], F32)
            ar_out = dram.tile([128, 4 * (D + 1)], F32)
            nc.gpsimd.dma_start(out=ar_in[:],
                                in_=pool_sb.rearrange("p a b -> p (a b)"))
            all_reduce(ar_in, ar_out)

        # ---------------- Phase B ----------------
        with contextlib.ExitStack() as phB:
            pBw = phB.enter_context(tc.tile_pool(name="pBw", bufs=1))
            pB = phB.enter_context(tc.tile_pool(name="pB", bufs=2))
            pBh = phB.enter_context(tc.tile_pool(name="pBh", bufs=3))
            psB = phB.enter_context(tc.tile_pool(name="psB", bufs=2, space="PSUM"))
            psBs = phB.enter_context(tc.tile_pool(name="psBs", bufs=1, space="PSUM"))

            # sf^T [d, h, m] head-major (all base 0); odd heads DMA-shifted down.
            # Covers only this core's sonata half (MH tokens).
            sfT = pBw.tile([64, H, MH], F32)
            for hd in range(4):  # hd chunk = heads 2hd, 2hd+1
                ps = psB.tile([128, MH], F32, tag="sf", bufs=1)
                for sd in range(6):
                    nc.tensor.matmul(
                        ps[:], lhsT=wsp_sb[:, sd, hd * 128 : (hd + 1) * 128],
                        rhs=snt_sb[:, sd, :], start=(sd == 0), stop=(sd == 5))
                nc.scalar.activation(out=sfT[:, 2 * hd, :], in_=ps[0:64, :],
                                     func=AF.Identity, bias=bsp_c[0:64, hd : hd + 1])
                up = pB.tile([128, MH], F32, tag="sfup")
                nc.scalar.activation(out=up[64:128, :], in_=ps[64:128, :],
                                     func=AF.Identity, bias=bsp_c[64:128, hd : hd + 1])
                nc.sync.dma_start(out=sfT[:, 2 * hd + 1, :], in_=up[64:128, :])

            # --- hoisted cross-attention K/V (independent of AllReduce 1) ---
            # These fill the PE pipeline while AR1 is in flight (in-order PE).
            NMC = MH // 128  # m-chunks on this core
            ksT_all = pBw.tile([64, H, MH], F32)
            vse_all = pBw.tile([128, H, NMC, D + 1], F32)
            nc.vector.memset(vse_all[:, :, :, D : D + 1], 1.0)
            for h in range(H):
                kp = psBs.tile([64, MH], F32, tag="small", bufs=3)
                nc.tensor.matmul(kp[:], lhsT=w64[:, 3, :], rhs=sfT[:, h, :],
                                 start=True, stop=True)
                nc.scalar.activation(out=ksT_all[:, h, :], in_=kp[:],
                                     func=AF.Identity, bias=bck_c[:])
                for mo in range(NMC):
                    vp = psBs.tile([128, 64], F32, tag="vp", bufs=2)
                    nc.tensor.matmul(vp[:],
                                     lhsT=sfT[:, h, mo * 128 : (mo + 1) * 128],
                                     rhs=w64[:, 4, :], start=True, stop=True)
                    if flags["bcv"]:
                        nc.vector.tensor_add(out=vse_all[:, h, mo, 0:D], in0=vp[:],
                                             in1=bcv_bc[:])
                    else:
                        nc.scalar.activation(out=vse_all[:, h, mo, 0:D], in_=vp[:],
                                             func=AF.Copy)

            pool_red = pB.tile([128, 4, D + 1], F32, tag="poolred")
            nc.sync.dma_start(out=pool_red.rearrange("p a b -> p (a b)"), in_=ar_out[:])

            # cross-attention numerator/denominator partials, packed like pool
            ocst = pBw.tile([128, 4, D + 1], F32)
            osT_all = pBw.tile([64, H, 64], F32)  # self-attn out^T per head
            for hp in range(4):
                pr = pool_red[:, hp, :]  # [128, 65]: heads 2hp (low), 2hp+1 (high)
                nrm = pBh.tile([128, 1], F32, tag="nrm")
                nc.vector.tensor_scalar_add(out=nrm[:], in0=pr[:, D : D + 1],
                                            scalar1=1e-5)
                nc.vector.reciprocal(out=nrm[:], in_=nrm[:])
                st2 = pBh.tile([128, D], F32, tag="st2")
                if flags["bfx"]:
                    for hh in range(2):
                        h = 2 * hp + hh
                        sl = slice(hh * 64, hh * 64 + 64)
                        nc.sync.dma_start(out=bfx_bc[sl, :],
                                          in_=_bcast_ap(bfx.ap(), 64, D, offset=h * D))
                    tmpb = pBh.tile([128, D], F32, tag="tmpb")
                    nc.vector.tensor_scalar_mul(out=tmpb[:], in0=bfx_bc[:],
                                                scalar1=pr[:, D : D + 1])
                    nc.vector.tensor_add(out=tmpb[:], in0=tmpb[:], in1=pr[:, 0:D])
                    nc.vector.tensor_scalar_mul(out=st2[:], in0=tmpb[:], scalar1=nrm[:])
                else:
                    nc.vector.tensor_scalar_mul(out=st2[:], in0=pr[:, 0:D],
                                                scalar1=nrm[:])

                stT_ps = psBs.tile([64, 128], F32, tag="small", bufs=3)
                nc.tensor.transpose(stT_ps[:], st2[:], ident[:])
                stT = pBh.tile([64, 128], F32, tag="stT")
                nc.scalar.activation(out=stT[:], in_=stT_ps[:], func=AF.Copy)

                qkvT = pBh.tile([64, 3, 128], F32, tag="qkvT")
                for idx in range(3):
                    qp = psBs.tile([64, 128], F32, tag="small", bufs=3)
                    nc.tensor.matmul(qp[:], lhsT=w64[:, idx, :], rhs=stT[:],
                                     start=True, stop=True)
                    nc.scalar.activation(out=qkvT[:, idx, :], in_=qp[:],
                                         func=AF.Identity,
                                         bias=bqv_c[:, idx : idx + 1])

                for hh in range(2):
                    h = 2 * hp + hh
                    hs = slice(hh * 64, hh * 64 + 64)
                    # --- slice self-attention (per head, base-0 [64,64]) ---
                    a_ps = psBs.tile([64, 64], F32, tag="small", bufs=3)
                    nc.tensor.matmul(a_ps[:], lhsT=qkvT[:, 0, hs], rhs=qkvT[:, 1, hs],
                                     start=True, stop=True)
                    ea = pBh.tile([64, 64], F32, tag="ea")
                    za = pBh.tile([64, 1], F32, tag="za")
                    nc.scalar.activation(out=ea[:], in_=a_ps[:], func=AF.Exp,
                                         accum_out=za[:])
                    nc.vector.reciprocal(out=za[:], in_=za[:])
                    pa = pBh.tile([64, 64], F32, tag="pa")
                    nc.vector.tensor_scalar_mul(out=pa[:], in0=ea[:], scalar1=za[:])

                    v_ps = psBs.tile([64, 64], F32, tag="small", bufs=3)
                    nc.tensor.transpose(v_ps[:], qkvT[:, 2, hs], ident[0:64, 0:64])
                    v_sb = pBh.tile([64, 64], F32, tag="v_sb")
                    nc.scalar.activation(out=v_sb[:], in_=v_ps[:], func=AF.Copy)
                    pat_ps = psBs.tile([64, 64], F32, tag="small", bufs=3)
                    nc.tensor.transpose(pat_ps[:], pa[:], ident[0:64, 0:64])
                    pat = pBh.tile([64, 64], F32, tag="pat")
                    nc.scalar.activation(out=pat[:], in_=pat_ps[:], func=AF.Copy)
                    osf_ps = psBs.tile([64, 64], F32, tag="small", bufs=3)
                    nc.tensor.matmul(osf_ps[:], lhsT=v_sb[:], rhs=pat[:],
                                     start=True, stop=True)
                    nc.scalar.activation(out=osT_all[:, h, :], in_=osf_ps[:],
                                         func=AF.Copy)

                    # --- sonata cross-attention (this core's half) ---
                    ct_ps = psB.tile([128, NMC, 64], F32, tag="ct")
                    for mo in range(NMC):
                        nc.tensor.matmul(ct_ps[:, mo, :],
                                         lhsT=ksT_all[:, h, mo * 128 : (mo + 1) * 128],
                                         rhs=osT_all[:, h, :], start=True, stop=True)
                    ecT = pBh.tile([128, NMC, 64], F32, tag="ecT")
                    nc.scalar.activation(out=ecT.rearrange("p a b -> p (a b)"),
                                         in_=ct_ps.rearrange("p a b -> p (a b)"),
                                         func=AF.Exp)
                    oc_ps = psBs.tile([64, 128], F32, tag="small", bufs=3)
                    for mo in range(NMC):
                        nc.tensor.matmul(oc_ps[:, 0 : D + 1], lhsT=ecT[:, mo, :],
                                         rhs=vse_all[:, h, mo, :],
                                         start=(mo == 0), stop=(mo == NMC - 1))
                    # pack partial numerator|denominator for the pair AllReduce
                    if hh == 0:
                        nc.scalar.activation(out=ocst[0:64, hp, :],
                                             in_=oc_ps[:, 0 : D + 1], func=AF.Copy)
                    else:
                        octmp = pBh.tile([64, D + 1], F32, tag="octmp")
                        nc.scalar.activation(out=octmp[:], in_=oc_ps[:, 0 : D + 1],
                                             func=AF.Copy)
                        nc.sync.dma_start(out=ocst[64:128, hp, :], in_=octmp[:])

            ar2_in = dram.tile([128, 4 * (D + 1)], F32)
            ar2_out = dram.tile([128, 4 * (D + 1)], F32)
            nc.gpsimd.dma_start(out=ar2_in[:],
                                in_=ocst.rearrange("p a b -> p (a b)"))
            all_reduce(ar2_in, ar2_out)
            ocred = pB.tile([128, 4, D + 1], F32, tag="ocred")
            nc.sync.dma_start(out=ocred.rearrange("p a b -> p (a b)"), in_=ar2_out[:])

            # finish cross-attention + OS per head pair
            for hp in range(4):
                oc2 = pBh.tile([128, D], F32, tag="oc2")
                zc = pBh.tile([128, 1], F32, tag="zc")
                nc.vector.reciprocal(out=zc[:], in_=ocred[:, hp, D : D + 1])
                nc.vector.tensor_scalar_mul(out=oc2[:], in0=ocred[:, hp, 0:D],
                                            scalar1=zc[:])
                # odd head lives on partitions 64-127; shift down for transpose
                oc_lo = pBh.tile([64, D], F32, tag="oc_lo")
                nc.sync.dma_start(out=oc_lo[:], in_=oc2[64:128, :])
                for hh in range(2):
                    h = 2 * hp + hh
                    src = oc2[0:64, :] if hh == 0 else oc_lo[:]
                    ocT_ps = psBs.tile([64, 64], F32, tag="small", bufs=3)
                    nc.tensor.transpose(ocT_ps[:], src, ident[0:64, 0:64])
                    osfT = pBh.tile([64, 64], F32, tag="osfT")
                    nc.vector.tensor_add(out=osfT[:], in0=ocT_ps[:],
                                         in1=osT_all[:, h, :])
                    osr_ps = psBs.tile([64, C], F32, tag="small", bufs=3)
                    nc.tensor.matmul(osr_ps[:], lhsT=osfT[:], rhs=wo_sb[:, h, :],
                                     start=True, stop=True)
                    os_fin = pBh.tile([64, C], F32, tag="os_fin")
                    if flags["bo"]:
                        nc.vector.tensor_scalar_mul(out=os_fin[:], in0=bo_bc[:],
                                                    scalar1=1.0 / H)
                        nc.vector.tensor_add(out=os_fin[:], in0=os_fin[:],
                                             in1=osr_ps[:])
                    else:
                        nc.scalar.activation(out=os_fin[:], in_=osr_ps[:],
                                             func=AF.Copy)
                    if hh == 0:
                        nc.vector.tensor_copy(out=os_sb[0:64, hp, :], in_=os_fin[:])
                    else:
                        nc.sync.dma_start(out=os_sb[64:128, hp, :], in_=os_fin[:])

        # ---------------- Phase C ----------------
        with contextlib.ExitStack() as phC:
            pC = phC.enter_context(tc.tile_pool(name="pC", bufs=2))
            pCl = phC.enter_context(tc.tile_pool(name="pCl", bufs=6))
            psC = phC.enter_context(tc.tile_pool(name="psC", bufs=3, space="PSUM"))
            y_re = y.ap().rearrange("(s j p) c -> s p j c", j=8, p=128)
            for si in range(NSUP):
                stg = pC.tile([128, 8, C], F32, tag="stg")
                for j in range(8):
                    i = si * 8 + j
                    if i < RES:
                        lh = eT[:, :, i * 128 : (i + 1) * 128]
                    else:
                        eTl = pCl.tile([128, 4, 128], F32, tag="eTl")
                        nc.sync.dma_start(out=eTl[:], in_=eTsp[:, i - RES, :, :])
                        lh = eTl
                    o_ps = psC.tile([128, C], F32, tag="o")
                    for cc in range(4):
                        nc.tensor.matmul(o_ps[:], lhsT=lh[:, cc, :],
                                         rhs=os_sb[:, cc, :],
                                         start=(cc == 0), stop=(cc == 3))
                    nc.scalar.activation(out=stg[:, j, :], in_=o_ps[:], func=AF.Copy)
                nc.sync.dma_start(out=y_re[si], in_=stg[:])

    nc.compile()
    return nc


_CACHE: dict = {}


def _get_nc(n_cores: int, T: int, flags_key: tuple):
    key = (n_cores, T, flags_key)
    if key not in _CACHE:
        flags = dict(zip(("bias_s", "bfx", "bcv", "bo"), flags_key))
        _CACHE[key] = _build(n_cores, T, flags)
    return _CACHE[key]


def prep_inputs(inputs: dict, n_cores: int, T: int):
    """Host-side prep: transposes, weight folding, per-core input maps."""
    f32 = np.float32
    x = np.asarray(inputs["x"], f32)
    snt = np.asarray(inputs["sonata_features"], f32)
    temp = np.asarray(inputs["temperature"], f32).reshape(H)
    Wx, bx = np.asarray(inputs["Wx"], f32), np.asarray(inputs["bx"], f32)
    Wfx, bfx = np.asarray(inputs["Wfx"], f32), np.asarray(inputs["bfx"], f32)
    Wsl, bsl = np.asarray(inputs["Wslice"], f32), np.asarray(inputs["bslice"], f32)
    Wq, bq = np.asarray(inputs["Wq"], f32), np.asarray(inputs["bq"], f32)
    Wk, bk = np.asarray(inputs["Wk"], f32), np.asarray(inputs["bk"], f32)
    Wv, bv = np.asarray(inputs["Wv"], f32), np.asarray(inputs["bv"], f32)
    Wsp, bsp = np.asarray(inputs["Wsp"], f32), np.asarray(inputs["bsp"], f32)
    Wck, bck = np.asarray(inputs["Wck"], f32), np.asarray(inputs["bck"], f32)
    Wcv, bcv = np.asarray(inputs["Wcv"], f32), np.asarray(inputs["bcv"], f32)
    Wo, bo = np.asarray(inputs["Wo"], f32), np.asarray(inputs["bo"], f32)

    Wxs = np.zeros((C, HG), f32)
    bias_s = np.zeros((HG,), f32)
    for h in range(H):
        Wxs[:, h * G : (h + 1) * G] = (Wx[:, h * D : (h + 1) * D] @ Wsl) / temp[h]
        bias_s[h * G : (h + 1) * G] = (bx[h * D : (h + 1) * D] @ Wsl + bsl) / temp[h]
    flags = {
        "bias_s": bool(np.any(bias_s != 0)),
        "bfx": bool(np.any(bfx != 0)),
        "bcv": bool(np.any(bcv != 0)),
        "bo": bool(np.any(bo != 0)),
    }
    shared = {
        "wxs": np.ascontiguousarray(Wxs),
        "wfx": np.ascontiguousarray(Wfx),
        "wsp": np.ascontiguousarray(Wsp),
        "wq": np.ascontiguousarray(Wq * SCALE),
        "wk": np.ascontiguousarray(Wk),
        "wv": np.ascontiguousarray(Wv),
        "wck": np.ascontiguousarray(Wck * SCALE),
        "wcv": np.ascontiguousarray(Wcv),
        "wo": np.ascontiguousarray(Wo),
        "bqv": np.ascontiguousarray(np.stack([bq * SCALE, bk, bv])),
        "bck": np.ascontiguousarray(bck * SCALE),
        "bsp": np.ascontiguousarray(bsp),
    }
    if flags["bias_s"]:
        shared["bias_s"] = bias_s
    if flags["bfx"]:
        shared["bfx"] = bfx
    if flags["bcv"]:
        shared["bcv"] = bcv
    if flags["bo"]:
        shared["bo"] = bo

    in_maps = []
    for c in range(n_cores):
        b, half = c // 2, c % 2
        xt_c = np.ascontiguousarray(x[b, half * T : (half + 1) * T, :].T)
        snt_c = np.ascontiguousarray(snt[b].T[:, half * MH : (half + 1) * MH])
        in_maps.append({"xt": xt_c, "snt": snt_c, **shared})
    return in_maps, flags


def run(inputs: dict, n_cores: int = 8, T: int = N // 2, **spmd_kwargs):
    in_maps, flags = prep_inputs(inputs, n_cores, T)
    nc = _get_nc(n_cores, T, tuple(flags.values()))
    res = run_bass_kernel_spmd(nc, in_maps, core_ids=list(range(n_cores)),
                               **spmd_kwargs)
    out = np.zeros((B, N, C), np.float32)
    for c in range(n_cores):
        b, half = c // 2, c % 2
        out[b, half * T : (half + 1) * T, :] = res.results[c]["y"]
    return out, res


def kernel(**inputs) -> np.ndarray:
    out, _ = run(inputs)
    return out


# revision 18
# speedup vs baseline: 5.8626x; 5.8626x over previous
"""Trainium2 Bass kernel for Enhanced Physics Attention with Sonata.

Contract: kernel(**inputs) takes FULL unsharded numpy inputs (as produced by
setup_inputs()) and returns the FULL [B, N, C] output. Internally shards
across 8 NeuronCores: core c handles batch c//2, token half c%2, and sonata
half c%2 (cross-attention partials). Two pairwise AllReduces: pooled slice
tokens after phase A, cross-attention numerators/denominators in phase B.

Math (validated vs reference, rel err ~1e-6, fp32 throughout):
  Host folds Wslice+temperature into Wxs (s = x @ Wxs), SCALE into Wq/Wck.
  Phase A (token-major): e = exp(s) [logits bounded ~3, no max needed],
    Z = grouped sum, pooling slice_token = e^T @ (fx * 1/Z | 1/Z) — the
    appended 1/Z column yields slice_norm for free.
  Phase B: slice self-attention (replicated) + sonata cross-attention over
    this core's sonata half; ones-column gives the softmax denominator, the
    pair AllReduce sums numerator+denominator over the full sonata.
  Phase C: out = e_hat @ (out_slice @ Wo), using PE-transposed e_hat kept
    half in SBUF / half spilled to DRAM.

Self-contained: hardcodes all shapes; does not read sibling files.
"""

import contextlib
import sys

try:
    import concourse  # noqa: F401
except ImportError:
    sys.path.insert(0, "/opt/trn_rl_repo")

import numpy as np

import concourse.bass as bass
import concourse.tile as tile
from concourse import bacc, mybir
from concourse.bass_utils import run_bass_kernel_spmd
from concourse.masks import make_identity

F32 = mybir.dt.float32
AF = mybir.ActivationFunctionType

# Problem shapes
B, N, C = 4, 16384, 256
H, D, G = 8, 64, 64
M, SD = 1024, 768
SCALE = D**-0.5
HG = H * G  # 512
HD = H * D  # 512
MH = M // 2  # sonata tokens per core (m-split across the pair)


def _bcast_ap(dram_ap: bass.AP, parts: int, n: int, offset: int = 0) -> bass.AP:
    """AP reading n contiguous DRAM floats, replicated across `parts` partitions."""
    return bass.AP(
        tensor=dram_ap.tensor,
        offset=dram_ap.offset + offset,
        ap=[[0, parts], [1, n]],
    )


def _fbcast(ap2d: bass.AP, rep: int) -> bass.AP:
    """[p, k] AP -> [p, k, rep] with step-0 innermost broadcast."""
    return bass.AP(tensor=ap2d.tensor, offset=ap2d.offset,
                   ap=[ap2d.ap[0], ap2d.ap[1], [0, rep]])


def _build(n_cores: int, T: int, flags: dict, no_collective: bool = False):
    """Build the per-core Bass module. T = tokens per core (multiple of 1024)."""
    assert T % 1024 == 0
    NSUP = T // 1024  # super-tiles (1 MiB x loads)
    NT = T // 128  # 128-token tiles
    RES = NT // 2  # tiles whose e-hat^T stays resident in SBUF

    nc = bacc.Bacc(
        "TRN2", target_bir_lowering=False, debug=False, num_devices=n_cores
    )

    # ---- DRAM I/O ----
    xt = nc.dram_tensor("xt", [C, T], F32, kind="ExternalInput")  # x slice, transposed
    snt = nc.dram_tensor("snt", [SD, MH], F32, kind="ExternalInput")  # sonata half, T
    wxs = nc.dram_tensor("wxs", [C, HG], F32, kind="ExternalInput")
    wfx = nc.dram_tensor("wfx", [C, HD], F32, kind="ExternalInput")
    wsp = nc.dram_tensor("wsp", [SD, HD], F32, kind="ExternalInput")
    wq = nc.dram_tensor("wq", [D, D], F32, kind="ExternalInput")
    wk = nc.dram_tensor("wk", [D, D], F32, kind="ExternalInput")
    wv = nc.dram_tensor("wv", [D, D], F32, kind="ExternalInput")
    wck = nc.dram_tensor("wck", [D, D], F32, kind="ExternalInput")
    wcv = nc.dram_tensor("wcv", [D, D], F32, kind="ExternalInput")
    wo = nc.dram_tensor("wo", [HD, C], F32, kind="ExternalInput")
    bqv = nc.dram_tensor("bqv", [3, D], F32, kind="ExternalInput")  # bq,bk,bv rows
    bck = nc.dram_tensor("bck", [D], F32, kind="ExternalInput")
    bsp = nc.dram_tensor("bsp", [HD], F32, kind="ExternalInput")
    if flags["bias_s"]:
        bias_s = nc.dram_tensor("bias_s", [HG], F32, kind="ExternalInput")
    if flags["bfx"]:
        bfx = nc.dram_tensor("bfx", [HD], F32, kind="ExternalInput")
    if flags["bcv"]:
        bcv = nc.dram_tensor("bcv", [D], F32, kind="ExternalInput")
    if flags["bo"]:
        bo = nc.dram_tensor("bo", [C], F32, kind="ExternalInput")
    y = nc.dram_tensor("y", [T, C], F32, kind="ExternalOutput")

    groups = [[2 * i, 2 * i + 1] for i in range(n_cores // 2)]

    def all_reduce(ar_in, ar_out):
        if no_collective:
            nc.gpsimd.dma_start(out=ar_out[:], in_=ar_in[:])
        else:
            nc.gpsimd.collective_compute(
                "AllReduce", mybir.AluOpType.add, replica_groups=groups,
                ins=[ar_in.opt()], outs=[ar_out.opt()])

    with tile.TileContext(nc) as tc, contextlib.ExitStack() as top:
        singles = top.enter_context(tc.tile_pool(name="singles", bufs=1))
        dram = top.enter_context(tc.tile_pool(name="dram", bufs=1, space="DRAM"))

        # ---- resident weights / inputs ----
        # Emission order matters: the in-order DMA queues must deliver wxs/wfx
        # and the first x super-tile before anything else so PE starts ASAP.
        wxs_sb = singles.tile([128, 2, HG], F32)
        wfx_sb = singles.tile([128, 2, HD], F32)
        wxs_re = wxs.ap().rearrange("(k p) n -> p k n", p=128)
        wfx_re = wfx.ap().rearrange("(k p) n -> p k n", p=128)
        # k0 chunks first so the very first matmuls can start sooner
        nc.sync.dma_start(out=wxs_sb[:, 0, :], in_=wxs_re[:, 0, :])
        nc.sync.dma_start(out=wfx_sb[:, 0, :], in_=wfx_re[:, 0, :])
        nc.sync.dma_start(out=wxs_sb[:, 1, :], in_=wxs_re[:, 1, :])
        nc.sync.dma_start(out=wfx_sb[:, 1, :], in_=wfx_re[:, 1, :])
        ident = singles.tile([128, 128], F32)
        make_identity(nc, ident[:])

        # tiles for deferred loads (DMAs emitted after the first super-tile)
        snt_sb = singles.tile([128, 6, MH], F32)
        wsp_sb = singles.tile([128, 6, HD], F32)
        w64 = singles.tile([64, 5, D], F32)  # wq,wk,wv,wck,wcv
        wo_sb = singles.tile([64, H, C], F32)

        def load_phaseB_weights():
            nc.sync.dma_start(out=snt_sb[:],
                              in_=snt.ap().rearrange("(k p) m -> p k m", p=128))
            nc.sync.dma_start(out=wsp_sb[:],
                              in_=wsp.ap().rearrange("(k p) n -> p k n", p=128))
            for idx, w in enumerate((wq, wk, wv, wck, wcv)):
                nc.sync.dma_start(out=w64[:, idx, :], in_=w.ap())
            nc.sync.dma_start(out=wo_sb[:],
                              in_=wo.ap().rearrange("(h d) c -> d h c", d=64))

        # e-hat^T: chunk c holds heads 2c,2c+1 stacked on partitions.
        eT = singles.tile([128, 4, RES * 128], F32)  # resident half
        eTsp = dram.tile([128, NT - RES, 4, 128], F32)  # spilled half, tile-major
        # OS (slice-token outputs @ Wo), chunk-packed like eT
        os_sb = singles.tile([128, 4, C], F32)

        # small per-partition bias columns
        bqv_c = singles.tile([64, 3], F32)
        nc.sync.dma_start(out=bqv_c[:], in_=bqv.ap().rearrange("q d -> d q"))
        bck_c = singles.tile([64, 1], F32)
        nc.sync.dma_start(out=bck_c[:], in_=bck.ap().rearrange("(d o) -> d o", o=1))
        bsp_c = singles.tile([128, 4], F32)
        nc.sync.dma_start(out=bsp_c[:], in_=bsp.ap().rearrange("(k p) -> p k", p=128))

        if flags["bias_s"]:
            bias_s_bc = singles.tile([128, HG], F32)
            nc.sync.dma_start(out=bias_s_bc[:], in_=_bcast_ap(bias_s.ap(), 128, HG))
        if flags["bcv"]:
            bcv_bc = singles.tile([128, D], F32)
            nc.sync.dma_start(out=bcv_bc[:], in_=_bcast_ap(bcv.ap(), 128, D))
        if flags["bo"]:
            bo_bc = singles.tile([64, C], F32)
            nc.sync.dma_start(out=bo_bc[:], in_=_bcast_ap(bo.ap(), 64, C))
        if flags["bfx"]:
            bfx_bc = singles.tile([128, D], F32)

        # ---------------- Phase A ----------------
        with contextlib.ExitStack() as phA:
            pA = phA.enter_context(tc.tile_pool(name="pA", bufs=3))
            pAx = phA.enter_context(tc.tile_pool(name="pAx", bufs=2))
            psA = phA.enter_context(tc.tile_pool(name="psA", bufs=2, space="PSUM"))
            psAcc = phA.enter_context(tc.tile_pool(name="psAcc", bufs=1, space="PSUM"))

            # head h: partitions (h%2)*64, accumulator tile h//2. Each accumulator
            # gets its own PSUM bank (own 2KB zero region) so the start=True of one
            # head pair cannot mark another pair's freshly-written bytes pending-zero.
            pool_ps = [psAcc.tile([128, 128], F32, name=f"poolacc{s}", tag=f"acc{s}")
                       for s in range(4)]

            xt_re = xt.ap().rearrange("(k p) n -> p k n", p=128)
            for si in range(NSUP):
                xt_sb = pAx.tile([128, 2, 1024], F32, tag="xt")
                sl = slice(si * 1024, (si + 1) * 1024)
                if si == 0:
                    nc.sync.dma_start(out=xt_sb[:, 0, :], in_=xt_re[:, 0, sl])
                    nc.sync.dma_start(out=xt_sb[:, 1, :], in_=xt_re[:, 1, sl])
                else:
                    nc.sync.dma_start(out=xt_sb[:], in_=xt_re[:, :, sl])
                for j in range(8):
                    i = si * 8 + j
                    tok = slice(j * 128, (j + 1) * 128)
                    s_ps = psA.tile([128, HG], F32, tag="s", bufs=1)
                    nc.tensor.matmul(s_ps[:], lhsT=xt_sb[:, 0, tok],
                                     rhs=wxs_sb[:, 0, :], start=True, stop=False)
                    nc.tensor.matmul(s_ps[:], lhsT=xt_sb[:, 1, tok],
                                     rhs=wxs_sb[:, 1, :], start=False, stop=True)
                    fx_ps = psA.tile([128, HD], F32, tag="fx")
                    nc.tensor.matmul(fx_ps[:], lhsT=xt_sb[:, 0, tok],
                                     rhs=wfx_sb[:, 0, :], start=True, stop=False)
                    nc.tensor.matmul(fx_ps[:], lhsT=xt_sb[:, 1, tok],
                                     rhs=wfx_sb[:, 1, :], start=False, stop=True)

                    e = pA.tile([128, H, G], F32, tag="e")
                    ef = e.rearrange("p a b -> p (a b)")
                    if flags["bias_s"]:
                        s_sb = pA.tile([128, HG], F32, tag="ssb")
                        nc.vector.tensor_add(out=s_sb[:], in0=s_ps[:], in1=bias_s_bc[:])
                        nc.scalar.activation(out=ef, in_=s_sb[:], func=AF.Exp)
                    else:
                        nc.scalar.activation(out=ef, in_=s_ps[:], func=AF.Exp)

                    z = pA.tile([128, H], F32, tag="z")
                    nc.vector.reduce_sum(out=z[:], in_=e[:], axis=mybir.AxisListType.X)
                    zr = pA.tile([128, H], F32, tag="zr")
                    nc.vector.reciprocal(out=zr[:], in_=z[:])
                    zrb = _fbcast(zr[:], G)  # [128, 8, (0x64)]

                    eh = pA.tile([128, H, G], F32, tag="eh")
                    nc.vector.tensor_tensor(out=eh[:], in0=e[:], in1=zrb,
                                            op=mybir.AluOpType.mult)
                    fxe = pA.tile([128, H, D + 1], F32, tag="fxe")
                    nc.vector.tensor_tensor(
                        out=fxe[:, :, 0:D],
                        in0=fx_ps.rearrange("p (a b) -> p a b", a=H), in1=zrb,
                        op=mybir.AluOpType.mult)
                    nc.vector.tensor_copy(out=fxe[:, :, D], in_=zr[:])

                    for h in range(H):
                        nc.tensor.matmul(
                            pool_ps[h // 2][(h % 2) * 64 : (h % 2) * 64 + 64, 0 : D + 1],
                            lhsT=e[:, h, :], rhs=fxe[:, h, :],
                            start=(i == 0), stop=(i == NT - 1), skip_group_check=True)

                    etp = psA.tile([128, 4, 128], F32, tag="etp", bufs=1)
                    ehf = eh.rearrange("p a b -> p (a b)")
                    for cc in range(4):
                        nc.tensor.transpose(etp[:, cc, :],
                                            ehf[:, cc * 128 : (cc + 1) * 128],
                                            ident[:])
                    if i < RES:
                        nc.scalar.activation(
                            out=eT[:, :, i * 128 : (i + 1) * 128], in_=etp[:],
                            func=AF.Copy)
                    else:
                        sp = pA.tile([128, 4, 128], F32, tag="spill")
                        nc.scalar.activation(out=sp[:], in_=etp[:], func=AF.Copy)
                        nc.sync.dma_start(out=eTsp[:, i - RES, :, :], in_=sp[:])

                if si == 0:
                    load_phaseB_weights()

            # pooled partials -> AllReduce across the pair
            pool_sb = pA.tile([128, 4, D + 1], F32, tag="poolsb")
            for s in range(4):
                nc.scalar.activation(out=pool_sb[:, s, :],
                                     in_=pool_ps[s][:, 0 : D + 1], func=AF.Copy)
            ar_in = dram.tile([128, 4 * (D + 1)], F32)
            ar_out = dram.tile([128, 4 * (D + 1)], F32)
            nc.gpsimd.dma_start(out=ar_in[:],
                                in_=pool_sb.rearrange("p a b -> p (a b)"))
            all_reduce(ar_in, ar_out)

        # ---------------- Phase B ----------------
        with contextlib.ExitStack() as phB:
            pBw = phB.enter_context(tc.tile_pool(name="pBw", bufs=1))
            pB = phB.enter_context(tc.tile_pool(name="pB", bufs=2))
            pBh = phB.enter_context(tc.tile_pool(name="pBh", bufs=3))
            psB = phB.enter_context(tc.tile_pool(name="psB", bufs=2, space="PSUM"))
            psBs = phB.enter_context(tc.tile_pool(name="psBs", bufs=1, space="PSUM"))

            # sf^T [d, h, m] head-major (all base 0); odd heads DMA-shifted down.
            # Covers only this core's sonata half (MH tokens).
            sfT = pBw.tile([64, H, MH], F32)
            for hd in range(4):  # hd chunk = heads 2hd, 2hd+1
                ps = psB.tile([128, MH], F32, tag="sf", bufs=1)
                for sd in range(6):
                    nc.tensor.matmul(
                        ps[:], lhsT=wsp_sb[:, sd, hd * 128 : (hd + 1) * 128],
                        rhs=snt_sb[:, sd, :], start=(sd == 0), stop=(sd == 5))
                nc.scalar.activation(out=sfT[:, 2 * hd, :], in_=ps[0:64, :],
                                     func=AF.Identity, bias=bsp_c[0:64, hd : hd + 1])
                up = pB.tile([128, MH], F32, tag="sfup")
                nc.scalar.activation(out=up[64:128, :], in_=ps[64:128, :],
                                     func=AF.Identity, bias=bsp_c[64:128, hd : hd + 1])
                nc.sync.dma_start(out=sfT[:, 2 * hd + 1, :], in_=up[64:128, :])

            # --- hoisted cross-attention K/V (independent of AllReduce 1) ---
            # These fill the PE pipeline while AR1 is in flight (in-order PE).
            NMC = MH // 128  # m-chunks on this core
            ksT_all = pBw.tile([64, H, MH], F32)
            vse_all = pBw.tile([128, H, NMC, D + 1], F32)
            nc.vector.memset(vse_all[:, :, :, D : D + 1], 1.0)
            for h in range(H):
                kp = psBs.tile([64, MH], F32, tag="small", bufs=3)
                nc.tensor.matmul(kp[:], lhsT=w64[:, 3, :], rhs=sfT[:, h, :],
                                 start=True, stop=True)
                nc.scalar.activation(out=ksT_all[:, h, :], in_=kp[:],
                                     func=AF.Identity, bias=bck_c[:])
                for mo in range(NMC):
                    vp = psBs.tile([128, 64], F32, tag="vp", bufs=2)
                    nc.tensor.matmul(vp[:],
                                     lhsT=sfT[:, h, mo * 128 : (mo + 1) * 128],
                                     rhs=w64[:, 4, :], start=True, stop=True)
                    if flags["bcv"]:
                        nc.vector.tensor_add(out=vse_all[:, h, mo, 0:D], in0=vp[:],
                                             in1=bcv_bc[:])
                    else:
                        nc.scalar.activation(out=vse_all[:, h, mo, 0:D], in_=vp[:],
                                             func=AF.Copy)

            pool_red = pB.tile([128, 4, D + 1], F32, tag="poolred")
            nc.sync.dma_start(out=pool_red.rearrange("p a b -> p (a b)"), in_=ar_out[:])

            # cross-attention numerator/denominator partials, packed like pool
            ocst = pBw.tile([128, 4, D + 1], F32)
            osT_all = pBw.tile([64, H, 64], F32)  # self-attn out^T per head
            for hp in range(4):
                pr = pool_red[:, hp, :]  # [128, 65]: heads 2hp (low), 2hp+1 (high)
                nrm = pBh.tile([128, 1], F32, tag="nrm")
                nc.vector.tensor_scalar_add(out=nrm[:], in0=pr[:, D : D + 1],
                                            scalar1=1e-5)
                nc.vector.reciprocal(out=nrm[:], in_=nrm[:])
                st2 = pBh.tile([128, D], F32, tag="st2")
                if flags["bfx"]:
                    for hh in range(2):
                        h = 2 * hp + hh
                        sl = slice(hh * 64, hh * 64 + 64)
                        nc.sync.dma_start(out=bfx_bc[sl, :],
                                          in_=_bcast_ap(bfx.ap(), 64, D, offset=h * D))
                    tmpb = pBh.tile([128, D], F32, tag="tmpb")
                    nc.vector.tensor_scalar_mul(out=tmpb[:], in0=bfx_bc[:],
                                                scalar1=pr[:, D : D + 1])
                    nc.vector.tensor_add(out=tmpb[:], in0=tmpb[:], in1=pr[:, 0:D])
                    nc.vector.tensor_scalar_mul(out=st2[:], in0=tmpb[:], scalar1=nrm[:])
                else:
                    nc.vector.tensor_scalar_mul(out=st2[:], in0=pr[:, 0:D],
                                                scalar1=nrm[:])

                stT_ps = psBs.tile([64, 128], F32, tag="small", bufs=3)
                nc.tensor.transpose(stT_ps[:], st2[:], ident[:])
                stT = pBh.tile([64, 128], F32, tag="stT")
                nc.scalar.activation(out=stT[:], in_=stT_ps[:], func=AF.Copy)

                qkvT = pBh.tile([64, 3, 128], F32, tag="qkvT")
                for idx in range(3):
                    qp = psBs.tile([64, 128], F32, tag="small", bufs=3)
                    nc.tensor.matmul(qp[:], lhsT=w64[:, idx, :], rhs=stT[:],
                                     start=True, stop=True)
                    nc.scalar.activation(out=qkvT[:, idx, :], in_=qp[:],
                                         func=AF.Identity,
                                         bias=bqv_c[:, idx : idx + 1])

                for hh in range(2):
                    h = 2 * hp + hh
                    hs = slice(hh * 64, hh * 64 + 64)
                    # --- slice self-attention (per head, base-0 [64,64]) ---
                    a_ps = psBs.tile([64, 64], F32, tag="small", bufs=3)
                    nc.tensor.matmul(a_ps[:], lhsT=qkvT[:, 0, hs], rhs=qkvT[:, 1, hs],
                                     start=True, stop=True)
                    ea = pBh.tile([64, 64], F32, tag="ea")
                    za = pBh.tile([64, 1], F32, tag="za")
                    nc.scalar.activation(out=ea[:], in_=a_ps[:], func=AF.Exp,
                                         accum_out=za[:])
                    nc.vector.reciprocal(out=za[:], in_=za[:])
                    pa = pBh.tile([64, 64], F32, tag="pa")
                    nc.vector.tensor_scalar_mul(out=pa[:], in0=ea[:], scalar1=za[:])

                    v_ps = psBs.tile([64, 64], F32, tag="small", bufs=3)
                    nc.tensor.transpose(v_ps[:], qkvT[:, 2, hs], ident[0:64, 0:64])
                    v_sb = pBh.tile([64, 64], F32, tag="v_sb")
                    nc.scalar.activation(out=v_sb[:], in_=v_ps[:], func=AF.Copy)
                    pat_ps = psBs.tile([64, 64], F32, tag="small", bufs=3)
                    nc.tensor.transpose(pat_ps[:], pa[:], ident[0:64, 0:64])
                    pat = pBh.tile([64, 64], F32, tag="pat")
                    nc.scalar.activation(out=pat[:], in_=pat_ps[:], func=AF.Copy)
                    osf_ps = psBs.tile([64, 64], F32, tag="small", bufs=3)
                    nc.tensor.matmul(osf_ps[:], lhsT=v_sb[:], rhs=pat[:],
                                     start=True, stop=True)
                    nc.scalar.activation(out=osT_all[:, h, :], in_=osf_ps[:],
                                         func=AF.Copy)

                    # --- sonata cross-attention (this core's half) ---
                    ct_ps = psB.tile([128, NMC, 64], F32, tag="ct")
                    for mo in range(NMC):
                        nc.tensor.matmul(ct_ps[:, mo, :],
                                         lhsT=ksT_all[:, h, mo * 128 : (mo + 1) * 128],
                                         rhs=osT_all[:, h, :], start=True, stop=True)
                    ecT = pBh.tile([128, NMC, 64], F32, tag="ecT")
                    nc.scalar.activation(out=ecT.rearrange("p a b -> p (a b)"),
                                         in_=ct_ps.rearrange("p a b -> p (a b)"),
                                         func=AF.Exp)
                    oc_ps = psBs.tile([64, 128], F32, tag="small", bufs=3)
                    for mo in range(NMC):
                        nc.tensor.matmul(oc_ps[:, 0 : D + 1], lhsT=ecT[:, mo, :],
                                         rhs=vse_all[:, h, mo, :],
                                         start=(mo == 0), stop=(mo == NMC - 1))
                    # pack partial numerator|denominator for the pair AllReduce
                    if hh == 0:
                        nc.scalar.activation(out=ocst[0:64, hp, :],
                                             in_=oc_ps[:, 0 : D + 1], func=AF.Copy)
                    else:
                        octmp = pBh.tile([64, D + 1], F32, tag="octmp")
                        nc.scalar.activation(out=octmp[:], in_=oc_ps[:, 0 : D + 1],
                                             func=AF.Copy)
                        nc.sync.dma_start(out=ocst[64:128, hp, :], in_=octmp[:])

            ar2_in = dram.tile([128, 4 * (D + 1)], F32)
            ar2_out = dram.tile([128, 4 * (D + 1)], F32)
            nc.gpsimd.dma_start(out=ar2_in[:],
                                in_=ocst.rearrange("p a b -> p (a b)"))
            all_reduce(ar2_in, ar2_out)
            ocred = pB.tile([128, 4, D + 1], F32, tag="ocred")
            nc.sync.dma_start(out=ocred.rearrange("p a b -> p (a b)"), in_=ar2_out[:])

            # finish cross-attention + OS per head pair
            for hp in range(4):
                oc2 = pBh.tile([128, D], F32, tag="oc2")
                zc = pBh.tile([128, 1], F32, tag="zc")
                nc.vector.reciprocal(out=zc[:], in_=ocred[:, hp, D : D + 1])
                nc.vector.tensor_scalar_mul(out=oc2[:], in0=ocred[:, hp, 0:D],
                                            scalar1=zc[:])
                # odd head lives on partitions 64-127; shift down for transpose
                oc_lo = pBh.tile([64, D], F32, tag="oc_lo")
                nc.sync.dma_start(out=oc_lo[:], in_=oc2[64:128, :])
                for hh in range(2):
                    h = 2 * hp + hh
                    src = oc2[0:64, :] if hh == 0 else oc_lo[:]
                    ocT_ps = psBs.tile([64, 64], F32, tag="small", bufs=3)
                    nc.tensor.transpose(ocT_ps[:], src, ident[0:64, 0:64])
                    osfT = pBh.tile([64, 64], F32, tag="osfT")
                    nc.vector.tensor_add(out=osfT[:], in0=ocT_ps[:],
                                         in1=osT_all[:, h, :])
                    osr_ps = psBs.tile([64, C], F32, tag="small", bufs=3)
                    nc.tensor.matmul(osr_ps[:], lhsT=osfT[:], rhs=wo_sb[:, h, :],
                                     start=True, stop=True)
                    os_fin = pBh.tile([64, C], F32, tag="os_fin")
                    if flags["bo"]:
                        nc.vector.tensor_scalar_mul(out=os_fin[:], in0=bo_bc[:],
                                                    scalar1=1.0 / H)
                        nc.vector.tensor_add(out=os_fin[:], in0=os_fin[:],
                                             in1=osr_ps[:])
                    else:
                        nc.scalar.activation(out=os_fin[:], in_=osr_ps[:],
                                             func=AF.Copy)
                    if hh == 0:
                        nc.vector.tensor_copy(out=os_sb[0:64, hp, :], in_=os_fin[:])
                    else:
                        nc.sync.dma_start(out=os_sb[64:128, hp, :], in_=os_fin[:])

        # ---------------- Phase C ----------------
        with contextlib.ExitStack() as phC:
            pC = phC.enter_context(tc.tile_pool(name="pC", bufs=2))
            pCl = phC.enter_context(tc.tile_pool(name="pCl", bufs=6))
            psC = phC.enter_context(tc.tile_pool(name="psC", bufs=3, space="PSUM"))
            y_re = y.ap().rearrange("(s j p) c -> s p j c", j=8, p=128)
            for si in range(NSUP):
                stg = pC.tile([128, 8, C], F32, tag="stg")
                for j in range(8):
                    i = si * 8 + j
                    if i < RES:
                        lh = eT[:, :, i * 128 : (i + 1) * 128]
                    else:
                        eTl = pCl.tile([128, 4, 128], F32, tag="eTl")
                        nc.sync.dma_start(out=eTl[:], in_=eTsp[:, i - RES, :, :])
                        lh = eTl
                    o_ps = psC.tile([128, C], F32, tag="o")
                    for cc in range(4):
                        nc.tensor.matmul(o_ps[:], lhsT=lh[:, cc, :],
                                         rhs=os_sb[:, cc, :],
                                         start=(cc == 0), stop=(cc == 3))
                    nc.scalar.activation(out=stg[:, j, :], in_=o_ps[:], func=AF.Copy)
                nc.sync.dma_start(out=y_re[si], in_=stg[:])

    nc.compile()
    return nc


_CACHE: dict = {}


def _get_nc(n_cores: int, T: int, flags_key: tuple):
    key = (n_cores, T, flags_key)
    if key not in _CACHE:
        flags = dict(zip(("bias_s", "bfx", "bcv", "bo"), flags_key))
        _CACHE[key] = _build(n_cores, T, flags)
    return _CACHE[key]


def prep_inputs(inputs: dict, n_cores: int, T: int):
    """Host-side prep: transposes, weight folding, per-core input maps."""
    f32 = np.float32
    x = np.asarray(inputs["x"], f32)
    snt = np.asarray(inputs["sonata_features"], f32)
    temp = np.asarray(inputs["temperature"], f32).reshape(H)
    Wx, bx = np.asarray(inputs["Wx"], f32), np.asarray(inputs["bx"], f32)
    Wfx, bfx = np.asarray(inputs["Wfx"], f32), np.asarray(inputs["bfx"], f32)
    Wsl, bsl = np.asarray(inputs["Wslice"], f32), np.asarray(inputs["bslice"], f32)
    Wq, bq = np.asarray(inputs["Wq"], f32), np.asarray(inputs["bq"], f32)
    Wk, bk = np.asarray(inputs["Wk"], f32), np.asarray(inputs["bk"], f32)
    Wv, bv = np.asarray(inputs["Wv"], f32), np.asarray(inputs["bv"], f32)
    Wsp, bsp = np.asarray(inputs["Wsp"], f32), np.asarray(inputs["bsp"], f32)
    Wck, bck = np.asarray(inputs["Wck"], f32), np.asarray(inputs["bck"], f32)
    Wcv, bcv = np.asarray(inputs["Wcv"], f32), np.asarray(inputs["bcv"], f32)
    Wo, bo = np.asarray(inputs["Wo"], f32), np.asarray(inputs["bo"], f32)

    Wxs = np.zeros((C, HG), f32)
    bias_s = np.zeros((HG,), f32)
    for h in range(H):
        Wxs[:, h * G : (h + 1) * G] = (Wx[:, h * D : (h + 1) * D] @ Wsl) / temp[h]
        bias_s[h * G : (h + 1) * G] = (bx[h * D : (h + 1) * D] @ Wsl + bsl) / temp[h]
    flags = {
        "bias_s": bool(np.any(bias_s != 0)),
        "bfx": bool(np.any(bfx != 0)),
        "bcv": bool(np.any(bcv != 0)),
        "bo": bool(np.any(bo != 0)),
    }
    shared = {
        "wxs": np.ascontiguousarray(Wxs),
        "wfx": np.ascontiguousarray(Wfx),
        "wsp": np.ascontiguousarray(Wsp),
        "wq": np.ascontiguousarray(Wq * SCALE),
        "wk": np.ascontiguousarray(Wk),
        "wv": np.ascontiguousarray(Wv),
        "wck": np.ascontiguousarray(Wck * SCALE),
        "wcv": np.ascontiguousarray(Wcv),
        "wo": np.ascontiguousarray(Wo),
        "bqv": np.ascontiguousarray(np.stack([bq * SCALE, bk, bv])),
        "bck": np.ascontiguousarray(bck * SCALE),
        "bsp": np.ascontiguousarray(bsp),
    }
    if flags["bias_s"]:
        shared["bias_s"] = bias_s
    if flags["bfx"]:
        shared["bfx"] = bfx
    if flags["bcv"]:
        shared["bcv"] = bcv
    if flags["bo"]:
        shared["bo"] = bo

    in_maps = []
    for c in range(n_cores):
        b, half = c // 2, c % 2
        xt_c = np.ascontiguousarray(x[b, half * T : (half + 1) * T, :].T)
        snt_c = np.ascontiguousarray(snt[b].T[:, half * MH : (half + 1) * MH])
        in_maps.append({"xt": xt_c, "snt": snt_c, **shared})
    return in_maps, flags


def run(inputs: dict, n_cores: int = 8, T: int = N // 2, **spmd_kwargs):
    in_maps, flags = prep_inputs(inputs, n_cores, T)
    nc = _get_nc(n_cores, T, tuple(flags.values()))
    res = run_bass_kernel_spmd(nc, in_maps, core_ids=list(range(n_cores)),
                               **spmd_kwargs)
    out = np.zeros((B, N, C), np.float32)
    for c in range(n_cores):
        b, half = c // 2, c % 2
        out[b, half * T : (half + 1) * T, :] = res.results[c]["y"]
    return out, res


def kernel(**inputs) -> np.ndarray:
    out, _ = run(inputs)
    return out
